# revision 15
# baseline (speedup 1.0000x reference)
"""Trainium2 Bass kernel for nn_AutoencoderDecoderLayer (S=1024, B=8, E=1024, NH=16, F=4096).

Strategy: data-parallel over batch B=8 -> one batch element per NeuronCore.
Per core one full decoder layer over (S=1024, E=1024) tokens.

Precision plan (validated against the fp32 reference with a numpy mirror):
  - Dense matmuls run in fp8(e4m3) using DoubleRow perf mode (0.5 PE
    cycles/output column for a 256-deep contraction = 4x the fp16 rate).
  - Weights are pre-scaled by 32 on the host (so their hi/lo split channels
    stay out of e4m3's subnormal range) and split into hi + lo fp8 pairs;
    the inverse scale rides the free psum-evacuation scale.
  - q/k/v, fc1 and fc2 are fully error-compensated (act hi/lo x weight
    hi/lo, dropping only the lo*lo term); wo and pgh compensate the weight
    side only; z projections are computed on the host in fp32.
  - Attention: scores contract k as an (hi,lo) DoubleRow pair against a
    broadcast q (stride-0 slot); probs/V accumulate token-tile pairs.
    exp outputs are fp8 with a global shift (softmax-invariant).
  - Residual stream and layernorm math are fp16/fp32.

Schedule: phase A (qkv+attention) mostly PE/Act bound; phases B..G are
pipelined over token halves so the DVE/Act-heavy LN/gate work of one half
overlaps the PE-heavy fc1/fc2 of the other.  Weight DMA is streamed ahead
of use (wo/pgh during A, fc1 during B, fc2 during F).
"""

import sys

sys.path.insert(0, "/opt/trn_rl_repo")

from contextlib import ExitStack

import numpy as np

import concourse.bass as bass
import concourse.mybir as mybir
import concourse.tile as tile
from concourse.masks import make_identity
from concourse.vector_clock import ScopedClock

P = 128
S, B, E, NH, F = 1024, 8, 1024, 16, 4096
HD = E // NH            # 64
TT = S // P             # 8 token tiles
KC2 = E // 256          # 4 contraction chunk-pairs over E
FKC2 = F // 256         # 16 chunk-pairs over F
ET = E // P             # 8 feature tiles
F1T = F // P            # 32 fc1 output tiles
NCH = 2                 # token-half pipeline chunks for B..G
CH = S // NCH           # 512
WS = 32.0               # host weight pre-scale
ESHIFT = -4.0           # exp shift (softmax-invariant); set vs max masked score
MASKVAL = -104.0        # additive causal mask (exact in e4m3)
DENEPS = 1e-6           # guards 1/den when an entire prob row flushes to 0

f32 = mybir.dt.float32
f16 = mybir.dt.float16
f8 = mybir.dt.float8e4
f8e5 = mybir.dt.float8e5

DR = mybir.MatmulPerfMode.DoubleRow
AF = mybir.ActivationFunctionType
OP = mybir.AluOpType

_MAX_DRAIN_WAITS = 1


def _split_drain_and_barrier(self, tick_clock, wait_clock):
    """This walrus build rejects >1 sem-wait on a CTRL Drain; split the final
    tile drain's wait list across a chain of Drains on the same engine."""
    drain_inst = self.nc.sync.drain()
    wait_clock.add_sem_waits(
        drain_inst.ins, ScopedClock({None: tick_clock.global_clock})
    )
    si = drain_inst.ins.sync_info
    if si is not None and len(si.on_wait) > _MAX_DRAIN_WAITS:
        waits = list(si.on_wait)
        drain_inst.ins.sync_info = mybir.SyncInfo(
            on_wait=waits[:_MAX_DRAIN_WAITS], on_update=list(si.on_update)
        )
        rest = waits[_MAX_DRAIN_WAITS:]
        for i in range(0, len(rest), _MAX_DRAIN_WAITS):
            extra = self.nc.sync.drain()
            extra.ins.sync_info = mybir.SyncInfo(
                on_wait=rest[i : i + _MAX_DRAIN_WAITS], on_update=[]
            )
    self.nc.all_engine_barrier()
    assert self.sems is not None
    popped = self.nc._tile_sem_poison_stack.pop()
    assert popped is self._sem_poison
    self.nc.clear_and_free_semaphores(list(self.sems.allocated().values()))
    self.nc.all_engine_barrier()


tile.TileContext._drain_and_barrier = _split_drain_and_barrier


def _split_waits_in_bir(bir_bytes):
    """This walrus build accepts at most ONE sem-wait per instruction.
    Hoist extra on_wait entries onto NoOp instructions inserted just before
    the owning instruction on the same engine (waits AND together, and each
    engine executes its stream in order, so this is semantics-preserving)."""
    import json

    d = json.loads(bir_bytes)
    cnt = 0

    def fix_block(blk):
        nonlocal cnt
        insts = blk.get("instructions") or []
        out = []
        for ins in insts:
            si = ins.get("sync_info")
            if si:
                waits = si.get("on_wait") or []
                if len(waits) > 1:
                    for w in waits[:-1]:
                        cnt += 1
                        out.append(
                            {
                                "name": f"wsplit-{cnt}",
                                "opcode": "NoOp",
                                "engine": ins["engine"],
                                "ins": [],
                                "outs": [],
                                "sync_info": {"on_wait": [w], "on_update": []},
                            }
                        )
                    si["on_wait"] = waits[-1:]
            out.append(ins)
        blk["instructions"] = out
        for sub in blk.get("blocks") or []:
            fix_block(sub)

    for fn in d.get("functions", []):
        for b in fn.get("blocks", []):
            fix_block(b)
    return json.dumps(d).encode()


def _install_bir_wait_split():
    from concourse import bass2jax, bass_utils

    if getattr(bass_utils, "_orig_compile_bir_kernel", None) is None:
        bass_utils._orig_compile_bir_kernel = bass_utils.compile_bir_kernel

        def patched(bir_json, tmpdir, neff_name="file.neff"):
            return bass_utils._orig_compile_bir_kernel(
                _split_waits_in_bir(bir_json), tmpdir, neff_name=neff_name
            )

        bass_utils.compile_bir_kernel = patched
        bass2jax.compile_bir_kernel = patched


_install_bir_wait_split()


def build_program(flags=("skipb",), reps=1):
    """flags: 'skipb' present -> ln b-vectors are all-zero and bo/fc2b/bv are
    zero, so their (token-broadcast) adds can be skipped."""
    skipb = "skipb" in flags
    nc = bass.Bass("TRN2", target_bir_lowering=False, debug=False, num_devices=1)

    def din(name, shape, dt):
        return nc.dram_tensor(name, shape, dt, kind="ExternalInput").ap()

    # activations
    x8h_d = din("x8h", (KC2 * P, 2, S), f8)
    x8l_d = din("x8l", (KC2 * P, 2, S), f8)
    x16_d = din("x16", (E, S), f16)
    zcols_d = din("zcols", (P, 2 * ET), f32)  # host: [zg cols | zv cols]
    # fp8 weights (pre-scaled x32, W.T chunk-pair layout [c*128+p, i, f])
    wname = {}
    for nm in ("wq", "wk", "wv", "wo", "pgh"):
        for hl in ("h", "l"):
            wname[nm + hl] = din(nm + hl, (KC2 * P, 2, E), f8)
    fc1h_d = din("fc1h", (KC2 * P, 2, F), f8)
    fc1l_d = din("fc1l", (KC2 * P, 2, F), f8)
    fc2h_d = din("fc2h", (FKC2 * P, 2, E), f8)
    fc2l_d = din("fc2l", (FKC2 * P, 2, E), f8)
    # packed per-feature columns: 11 x [P, ET] + fc1 [P, F1T] = [P, 120]
    cols_d = din("colpack", (P, 11 * ET + F1T), f32)
    cmask_d = din("cmask8", (P, P), f8)
    out = nc.dram_tensor("out", (E, S), f32, kind="ExternalOutput").ap()

    with tile.TileContext(nc) as tc, ExitStack() as top:
        pool = lambda st, nm, bufs, **kw: st.enter_context(
            tc.tile_pool(name=nm, bufs=bufs, **kw)
        )
        const = pool(top, "const", 1, side="left")

        # ---------------- constants ----------------
        ident16 = const.tile([P, P], f16, name="ident16")
        make_identity(nc, ident16)
        ident32 = const.tile([P, P], f32, name="ident32")
        make_identity(nc, ident32)
        maskz = const.tile([P, 2, P], f8, name="maskz")  # mask slot0, zero slot1
        nc.vector.memset(maskz, 0.0)
        nc.sync.dma_start(maskz[:, 0, :], cmask_d)
        identz = const.tile([P, 2, P], f8, name="identz")
        nc.vector.memset(identz, 0.0)
        make_identity(nc, identz[:, 0, :])
        ones1 = const.tile([P, 1], f16, name="ones1")
        nc.vector.memset(ones1, 1.0)
        ones_row = const.tile([1, P], f16, name="ones_row")
        nc.vector.memset(ones_row, 1.0)
        eps_t = const.tile([P, 1], f32, name="eps_t")
        nc.vector.memset(eps_t, 1e-5)
        eshift_t = const.tile([P, 1], f32, name="eshift_t")
        nc.vector.memset(eshift_t, ESHIFT)

        colpack = const.tile([P, 11 * ET + F1T], f32, name="colpack_sb")
        nc.sync.dma_start(colpack, cols_d)
        _c = [colpack[:, i * ET : (i + 1) * ET] for i in range(11)]
        bqc, bkc, bvc, boc, fc2bc = _c[0], _c[1], _c[2], _c[3], _c[4]
        gc = _c[5:8]
        bc = _c[8:11]
        fc1bc = colpack[:, 11 * ET : 11 * ET + F1T]

        zgv = const.tile([P, 2, ET], f32, name="zgv_sb")
        nc.sync.dma_start(
            zgv, zcols_d.rearrange("p (j e) -> p j e", j=2)
        )

        def emit_layer(rep):
            lay = ExitStack()
            stat_sb = pool(lay, "stat_sb", 2, side="left")  # small stats
            keep = pool(lay, "keep", 1, side="left")        # x2 + x2_8 [B..G]

            # ---------- LN helper (feature-major) ----------
            def layer_norm_fm(rtiles, g_col, b_col, nm, out_tiles, t0=0, t1=S,
                              ot0=None, post_et=None):
                """Feature-major LN over rtiles[:, t0:t1); writes
                out_tiles[:, ot0:ot0+nt) (ot0 defaults to t0)."""
                nt = t1 - t0
                if ot0 is None:
                    ot0 = t0
                ntb = nt // P
                lns = ExitStack()
                ps_st = lns.enter_context(
                    tc.tile_pool(name=f"ps_{nm}", bufs=1, space="PSUM")
                )
                sq_p = lns.enter_context(
                    tc.tile_pool(name=f"sq_{nm}", bufs=2, side="right")
                )
                xq = ps_st.tile([P, 2, ntb], f32, tag="xq", name=f"xq_{nm}")
                xs = xq[:, 0, :]
                qs = xq[:, 1, :]
                for et in range(ET):
                    sq = sq_p.tile([P, nt], f16, tag="sq", bufs=2, name=f"sq_{nm}_{et}")
                    nc.vector.tensor_tensor(
                        sq, rtiles[et][:, t0:t1], rtiles[et][:, t0:t1], OP.mult
                    )
                    for tb in range(ntb):
                        rsl = rtiles[et][:, t0 + tb * P : t0 + (tb + 1) * P]
                        nc.tensor.matmul(
                            xs[:, tb : tb + 1], rsl, ones1,
                            start=(et == 0 and tb == 0),
                            stop=False,
                            skip_group_check=True,
                        )
                        nc.tensor.matmul(
                            qs[:, tb : tb + 1], sq[:, tb * P : (tb + 1) * P],
                            ones1,
                            start=False,
                            stop=(et == ET - 1 and tb == ntb - 1),
                            skip_group_check=True,
                        )
                mu = stat_sb.tile([P, ntb], f32, tag="mu", name=f"mu_{nm}")
                nc.vector.tensor_scalar(
                    mu, xs, scalar1=1.0 / E, scalar2=None, op0=OP.mult
                )
                msq = stat_sb.tile([P, ntb], f32, tag="msq", name=f"msq_{nm}")
                nc.vector.tensor_tensor(msq, mu, mu, OP.mult)
                var = stat_sb.tile([P, ntb], f32, tag="var", name=f"var_{nm}")
                nc.vector.scalar_tensor_tensor(
                    var, qs, 1.0 / E, msq, op0=OP.mult, op1=OP.subtract
                )
                sd = stat_sb.tile([P, ntb], f32, tag="sd", name=f"sd_{nm}")
                nc.scalar.activation(sd, var, AF.Sqrt, bias=eps_t, scale=1.0)
                st16 = stat_sb.tile([P, 2, ntb], f16, tag="st16", name=f"st16_{nm}")
                with nc.allow_low_precision(reason="fp16 rstd is ample for LN"):
                    nc.vector.reciprocal(st16[:, 0, :], sd)
                nc.vector.scalar_tensor_tensor(
                    st16[:, 1, :], mu, -1.0, st16[:, 0, :], op0=OP.mult, op1=OP.mult
                )
                pr = ps_st.tile([33, nt], f16, tag="str", name=f"str_{nm}")
                for b in range(ntb):
                    nc.tensor.transpose(
                        pr[0:1, b * P : (b + 1) * P], st16[:, 0, b : b + 1], ident16
                    )
                    nc.tensor.transpose(
                        pr[32:33, b * P : (b + 1) * P], st16[:, 1, b : b + 1], ident16
                    )
                row_r = stat_sb.tile([1, nt], f16, tag="rowr", name=f"rowr_{nm}")
                nc.scalar.activation(row_r, pr[0:1, :], AF.Copy)
                row_n = stat_sb.tile([1, nt], f16, tag="rown", name=f"rown_{nm}")
                nc.scalar.activation(row_n, pr[32:33, :], AF.Copy)
                # replicate rows across partitions: ones-col matmul -> psum -> f16
                rstd_bc = stat_sb.tile([P, nt], f16, tag="rbc", name=f"rbc_{nm}")
                nmur_bc = stat_sb.tile([P, nt], f16, tag="nbc", name=f"nbc_{nm}")
                for row, bcst in ((row_r, rstd_bc), (row_n, nmur_bc)):
                    for halfn in range(0, nt, 512):
                        nn = min(512, nt - halfn)
                        pbc = ps_st.tile([P, 512], f32, tag="pbc", name=f"pbc_{nm}_{halfn}")
                        nc.tensor.matmul(
                            pbc[:, 0:nn], ones_row,
                            row[:, halfn : halfn + nn],
                            start=True, stop=True,
                        )
                        nc.vector.tensor_copy(out=bcst[:, halfn : halfn + nn], in_=pbc[:, 0:nn])
                for et in range(ET):
                    osl = slice(ot0, ot0 + nt)
                    t = sq_p.tile([P, nt], f16, tag="tn", name=f"tn_{nm}_{et}")
                    nc.vector.scalar_tensor_tensor(
                        t, rtiles[et][:, t0:t1], g_col[:, et : et + 1], rstd_bc,
                        op0=OP.mult, op1=OP.mult,
                    )
                    nc.vector.scalar_tensor_tensor(
                        out_tiles[et][:, osl], nmur_bc, g_col[:, et : et + 1], t,
                        op0=OP.mult, op1=OP.add,
                    )
                    if not skipb:
                        nc.vector.tensor_scalar(
                            out_tiles[et][:, osl], out_tiles[et][:, osl],
                            scalar1=b_col[:, et : et + 1], scalar2=None,
                            op0=OP.add,
                        )
                    if post_et is not None:
                        post_et(et)
                lns.close()

            # attention outputs + wo/pgh weights (left side; freed at layer end)
            attw = pool(lay, "attw", 1, side="left")
            attnT8 = [
                attw.tile([P, 2, S], f8, name=f"attnT8_{c}") for c in range(KC2)
            ]
            wo_h, wo_l, pgh_h, pgh_l = [], [], [], []
            for c in range(KC2):
                for nm, lst in (
                    ("woh", wo_h), ("wol", wo_l), ("pghh", pgh_h), ("pghl", pgh_l)
                ):
                    lst.append(attw.tile([P, 2, E], f8, name=f"{nm}_{c}"))

            # ---------- Phase A: qkv + attention, interleaved per f-tile -----
            with ExitStack() as phA:
                xin = pool(phA, "xin", 1, side="right")
                wq_pool = pool(phA, "wq_pool", 1, side="right")

                # x first (first matmul needs it), then qkv weights, then the
                # wo/pgh prefetch into the longer-lived attw pool.
                x8h, x8l = [], []
                for c in range(KC2):
                    th = xin.tile([P, 2, S], f8, name=f"x8h_{c}")
                    nc.sync.dma_start(th, x8h_d[c * P : (c + 1) * P, :, :])
                    x8h.append(th)
                for c in range(KC2):
                    tl = xin.tile([P, 2, S], f8, name=f"x8l_{c}")
                    nc.sync.dma_start(tl, x8l_d[c * P : (c + 1) * P, :, :])
                    x8l.append(tl)
                qkv_w = {}
                for nm in ("wqh", "wql", "wkh", "wkl", "wvh", "wvl"):
                    tiles = []
                    for c in range(KC2):
                        t = wq_pool.tile([P, 2, E], f8, name=f"{nm}_{c}")
                        nc.sync.dma_start(t, wname[nm][c * P : (c + 1) * P, :, :])
                        tiles.append(t)
                    qkv_w[nm] = tiles
                for c in range(KC2):
                    for nm, lst in (
                        ("woh", wo_h), ("wol", wo_l), ("pghh", pgh_h), ("pghl", pgh_l)
                    ):
                        nc.sync.dma_start(
                            lst[c], wname[nm][c * P : (c + 1) * P, :, :]
                        )

                asb = pool(phA, "asb", 1, side="right")
                expp = pool(phA, "expp", 12, side="right")
                a16p = pool(phA, "a16p", 2, side="right")
                psA = pool(phA, "psA", 1, space="PSUM")

                q8 = [
                    asb.tile([P, S], f8, tag="q8", bufs=3, name=f"q8_{et}")
                    for et in range(ET)
                ]
                k8 = [
                    asb.tile([P, 2, S], f8, tag="k8", bufs=3, name=f"k8_{et}")
                    for et in range(ET)
                ]
                v1p = [
                    asb.tile([P, 2, NH, HD + 1], f8, name=f"v1p_{c}")
                    for c in range(KC2)
                ]
                for c in range(KC2):
                    nc.vector.memset(v1p[c][:, :, :, HD : HD + 1], 1.0)

                def ps_half(nm):
                    return psA.tile([P, 512], f32, tag="pA", bufs=2, name=nm)

                def ps_sc(nm):
                    return psA.tile([P, 512], f32, tag="sc", bufs=4, name=nm)

                ep = {}  # h -> list of 4 pair tiles (ring keeps ~3 heads)

                def proj_fm(et, whi, wlo, nm, evac):
                    for tb in range(2):
                        rs = slice(tb * 512, (tb + 1) * 512)
                        ps = ps_half(f"{nm}_ps{et}_{tb}")
                        steps = []
                        for c in range(KC2):
                            lh = whi[c][:, :, et * P : (et + 1) * P]
                            ll = wlo[c][:, :, et * P : (et + 1) * P]
                            steps.append((lh, x8h[c][:, :, rs]))
                            steps.append((ll, x8h[c][:, :, rs]))
                            steps.append((lh, x8l[c][:, :, rs]))
                        for i, (lhsT, rhs) in enumerate(steps):
                            nc.tensor.matmul(
                                ps, lhsT, rhs,
                                start=(i == 0), stop=(i == len(steps) - 1),
                                perf_mode=DR,
                            )
                        evac(ps, rs)

                def scores_head(h):
                    et = h // 2
                    r0 = (h % 2) * HD
                    eps_tiles = [
                        expp.tile([P, 2, S], f8e5 if c == 0 else f8,
                                  tag="ep5" if c == 0 else "ep",
                                  bufs=4 if c == 0 else 11,
                                  name=f"ep{h}_{c}")
                        for c in range(KC2)
                    ]
                    ep[h] = eps_tiles
                    for tjt in range(TT):
                        base = tjt * P
                        lhsT = k8[et][r0 : r0 + HD, :, base : base + P]
                        off = base
                        while off < S:
                            n = min(512 - (off % 512), S - off)
                            ps = ps_sc(f"sc{h}_{tjt}_{off}")
                            rhs = q8[et][
                                r0 : r0 + HD, None, off : off + n
                            ].to_broadcast([HD, 2, n])
                            nc.tensor.matmul(
                                ps[:, 0:n], lhsT, rhs,
                                start=True, stop=(off != base), perf_mode=DR,
                            )
                            if off == base:
                                nc.tensor.matmul(
                                    ps[:, 0:P], identz, maskz,
                                    start=False, stop=True, perf_mode=DR,
                                )
                            nc.scalar.activation(
                                eps_tiles[tjt // 2][:, tjt % 2, off : off + n],
                                ps[:, 0:n], AF.Exp, bias=eshift_t, scale=1.0,
                            )
                            off += n

                def av_pair(hp):
                    """probs @ V + evac + transpose for heads 2hp, 2hp+1."""
                    a16 = a16p.tile([P, 2, HD, TT], f16, tag="a16", name=f"a16_{hp}")
                    for tit in range(TT):
                        pav = psA.tile(
                            [P, 2, HD + 1], f32, tag="pav", bufs=1,
                            name=f"pav{hp}_{tit}",
                        )
                        for hh in range(2):
                            h = hp * 2 + hh
                            ept = ep[h]
                            npair = tit // 2
                            for c in range(npair):
                                nc.tensor.matmul(
                                    pav[:, hh, :],
                                    ept[c][:, :, tit * P : (tit + 1) * P],
                                    v1p[c][:, :, h, :],
                                    start=(c == 0), stop=False, perf_mode=DR,
                                )
                            for tj in range(2 * npair, tit + 1):
                                nc.tensor.matmul(
                                    pav[:, hh, :],
                                    ept[tj // 2][:, tj % 2, tit * P : (tit + 1) * P],
                                    v1p[tj // 2][:, tj % 2, h, :],
                                    start=(tj == 0), stop=(tj == tit),
                                )
                        den = stat_sb.tile([P, 2], f32, tag="den", name=f"den{hp}_{tit}")
                        nc.vector.tensor_scalar(
                            den, pav[:, :, HD], scalar1=DENEPS, scalar2=None,
                            op0=OP.add,
                        )
                        rc = stat_sb.tile([P, 2], f32, tag="rc", name=f"rc{hp}_{tit}")
                        nc.vector.reciprocal(rc, den)
                        nc.vector.tensor_tensor(
                            a16[:, :, :, tit],
                            pav[:, :, 0:HD],
                            rc[:, :, None].to_broadcast([P, 2, HD]),
                            OP.mult,
                        )
                    # transpose: a16 [tok, (2 heads, d), tt] -> attnT8 FM slot et=hp
                    for tt in range(TT):
                        pt = psA.tile([P, P], f16, tag="tr", bufs=1, name=f"trA{hp}_{tt}")
                        nc.tensor.transpose(
                            pt,
                            a16[:, :, :, tt].rearrange("p h d -> p (h d)"),
                            ident16,
                        )
                        dstT = attnT8[hp // 2][:, hp % 2, tt * P : (tt + 1) * P]
                        nc.vector.tensor_copy(out=dstT, in_=pt)

                for et in range(ET):
                    def evac_q(ps, rs, et=et):
                        nc.vector.tensor_scalar(
                            q8[et][:, rs], ps, 1.0 / (WS * 8.0),
                            bqc[:, et : et + 1], op0=OP.mult, op1=OP.add,
                        )
                    proj_fm(et, qkv_w["wqh"], qkv_w["wql"], "q", evac_q)

                    def evac_k(ps, rs, et=et):
                        nc.vector.tensor_scalar(
                            k8[et][:, 0, rs], ps, 1.0 / WS,
                            bkc[:, et : et + 1], op0=OP.mult, op1=OP.add,
                        )
                        nc.vector.scalar_tensor_tensor(
                            k8[et][:, 1, rs], ps, 1.0 / WS, k8[et][:, 0, rs],
                            op0=OP.mult, op1=OP.subtract,
                        )
                    proj_fm(et, qkv_w["wkh"], qkv_w["wkl"], "k", evac_k)
                    if et % 4 == 0:
                        c4 = et // 4
                        for tt in range(TT):
                            psv = ps_half(f"v_ps{c4}_{tt}")
                            steps = []
                            for c in range(KC2):
                                lh = x8h[c][:, :, tt * P : (tt + 1) * P]
                                ll = x8l[c][:, :, tt * P : (tt + 1) * P]
                                wh = qkv_w["wvh"][c][:, :, c4 * 512 : (c4 + 1) * 512]
                                wl = qkv_w["wvl"][c][:, :, c4 * 512 : (c4 + 1) * 512]
                                steps += [(lh, wh), (ll, wh), (lh, wl)]
                            for i, (lhsT, rhs) in enumerate(steps):
                                nc.tensor.matmul(
                                    psv, lhsT, rhs,
                                    start=(i == 0), stop=(i == len(steps) - 1),
                                    perf_mode=DR,
                                )
                            nc.vector.tensor_scalar(
                                v1p[tt // 2][:, tt % 2, c4 * 8 : (c4 + 1) * 8, 0:HD],
                                psv.rearrange("p (h d) -> p h d", d=HD),
                                1.0 / WS, bvc[:, et : et + 1],
                                op0=OP.mult, op1=OP.add,
                            )
                    scores_head(2 * et)
                    scores_head(2 * et + 1)
                    if et >= 1:
                        av_pair(et - 1)
                av_pair(ET - 1)

            # ---------- Phases B..G: token-half pipeline ---------------------
            x2 = [keep.tile([P, S], f16, name=f"x2_{et}") for et in range(ET)]
            x2_8h = [keep.tile([P, 2, S], f8, name=f"x2_8h_{c}") for c in range(KC2)]
            x2_8l = [keep.tile([P, 2, S], f8, name=f"x2_8l_{c}") for c in range(KC2)]

            # fc1 weights resident [B..F]; DMA streams during phase B.
            phF_st = ExitStack()
            wF = pool(phF_st, "wF", 1, side="right")
            f1_h = [[None] * KC2 for _ in range(4)]
            f1_l = [[None] * KC2 for _ in range(4)]
            for fb in range(4):
                for c in range(KC2):
                    th = wF.tile([P, 2, 8 * P], f8, name=f"f1h_{fb}_{c}")
                    nc.sync.dma_start(
                        th,
                        fc1h_d[c * P : (c + 1) * P, :, fb * 1024 : (fb + 1) * 1024],
                    )
                    f1_h[fb][c] = th
                    tl = wF.tile([P, 2, 8 * P], f8, name=f"f1l_{fb}_{c}")
                    nc.sync.dma_start(
                        tl,
                        fc1l_d[c * P : (c + 1) * P, :, fb * 1024 : (fb + 1) * 1024],
                    )
                    f1_l[fb][c] = tl

            # ----- phase B per token half: o-proj + LN1 + gate + LN2 --------
            with ExitStack() as phB:
                psB = pool(phB, "psB", 1, space="PSUM")
                for half in range(NCH):
                    hs = slice(half * CH, (half + 1) * CH)
                    bh = ExitStack()
                    bsb = pool(bh, f"bsb{half}", 1, side="right")
                    x16c = []
                    for et in range(ET):
                        t = bsb.tile([P, CH], f16, name=f"x16_{half}_{et}")
                        nc.sync.dma_start(t, x16_d[et * P : (et + 1) * P, hs])
                        x16c.append(t)
                    r1 = [bsb.tile([P, CH], f16, name=f"r1_{half}_{et}") for et in range(ET)]
                    for et in range(ET):
                        ps = psB.tile([P, CH], f32, tag="big", bufs=4, name=f"o_ps{half}_{et}")
                        steps = []
                        for c in range(KC2):
                            steps.append((wo_h[c][:, :, et * P : (et + 1) * P],
                                          attnT8[c][:, :, hs]))
                            steps.append((wo_l[c][:, :, et * P : (et + 1) * P],
                                          attnT8[c][:, :, hs]))
                        for i, (lhsT, rhs) in enumerate(steps):
                            nc.tensor.matmul(
                                ps, lhsT, rhs,
                                start=(i == 0), stop=(i == len(steps) - 1),
                                perf_mode=DR,
                            )
                        if not skipb:
                            tmp = bsb.tile([P, CH], f32, tag="botmp", name=f"bo_{half}_{et}")
                            nc.vector.tensor_scalar(
                                tmp, ps, scalar1=1.0 / WS, op0=OP.mult,
                                scalar2=boc[:, et : et + 1], op1=OP.add,
                            )
                            nc.vector.tensor_tensor(r1[et], tmp, x16c[et], OP.add)
                        elif et % 2 == 0:
                            nc.vector.scalar_tensor_tensor(
                                r1[et], ps, 1.0 / WS, x16c[et], op0=OP.mult, op1=OP.add
                            )
                        else:
                            # balance: Act evac + gpsimd residual add
                            tmp = bsb.tile([P, CH], f16, tag="botmp", name=f"bo_{half}_{et}")
                            nc.scalar.activation(tmp, ps, AF.Copy, scale=1.0 / WS)
                            nc.gpsimd.tensor_tensor(r1[et], tmp, x16c[et], OP.add)

                    x1 = [bsb.tile([P, CH], f16, name=f"x1_{half}_{et}") for et in range(ET)]
                    layer_norm_fm(r1, gc[0], bc[0], f"ln1_{half}", x1, t0=0, t1=CH)

                    x1_8 = [bsb.tile([P, 2, CH], f8, name=f"x1_8_{half}_{c}") for c in range(KC2)]
                    for et in range(ET):
                        dst = x1_8[et // 2][:, et % 2, :]
                        if et % 3 == 0:
                            nc.scalar.activation(dst, x1[et], AF.Copy)
                        elif et % 3 == 1:
                            nc.vector.tensor_copy(out=dst, in_=x1[et])
                        else:
                            nc.gpsimd.tensor_copy(out=dst, in_=x1[et])

                    r2 = [bsb.tile([P, CH], f16, name=f"r2_{half}_{et}") for et in range(ET)]
                    for et in range(ET):
                        ps = psB.tile([P, CH], f32, tag="big", bufs=4, name=f"g_ps{half}_{et}")
                        steps = []
                        for c in range(KC2):
                            steps.append((pgh_h[c][:, :, et * P : (et + 1) * P],
                                          x1_8[c][:, :, :]))
                            steps.append((pgh_l[c][:, :, et * P : (et + 1) * P],
                                          x1_8[c][:, :, :]))
                        for i, (lhsT, rhs) in enumerate(steps):
                            nc.tensor.matmul(
                                ps, lhsT, rhs,
                                start=(i == 0), stop=(i == len(steps) - 1),
                                perf_mode=DR,
                            )
                        gate = bsb.tile([P, CH], f16, tag="gate", name=f"gate_{half}_{et}")
                        nc.scalar.activation(
                            gate, ps, AF.Sigmoid,
                            bias=zgv[:, 0, et : et + 1], scale=1.0 / WS,
                        )
                        nc.vector.scalar_tensor_tensor(
                            r2[et], gate, zgv[:, 1, et : et + 1], x1[et],
                            op0=OP.mult, op1=OP.add,
                        )

                    layer_norm_fm(r2, gc[1], bc[1], f"ln2_{half}", x2,
                                  t0=0, t1=CH, ot0=half * CH)
                    for et in range(ET):
                        hslot = x2_8h[et // 2][:, et % 2, hs]
                        if et % 2 == 0:
                            nc.scalar.activation(hslot, x2[et][:, hs], AF.Copy)
                        else:
                            nc.gpsimd.tensor_copy(out=hslot, in_=x2[et][:, hs])
                        enl = nc.vector if et % 2 == 0 else nc.gpsimd
                        enl.tensor_tensor(
                            x2_8l[et // 2][:, et % 2, hs], x2[et][:, hs], hslot,
                            OP.subtract,
                        )
                    bh.close()

            # ----- phases F (fc1) + G (fc2 + LN3), interleaved per half -----
            wg_st = ExitStack()
            wG = pool(wg_st, "wG", 4, side="right")
            for half in range(NCH):
                hs = slice(half * CH, (half + 1) * CH)
                fh = ExitStack()
                h8p = pool(fh, f"h8p{half}", 1, side="right")
                h8h = [h8p.tile([P, 2, CH], f8, name=f"h8h_{half}_{c}")
                       for c in range(FKC2)]
                h8l = [h8p.tile([P, 2, CH], f8, name=f"h8l_{half}_{c}")
                       for c in range(FKC2)]
                with ExitStack() as phF:
                    h16p = pool(phF, f"h16p{half}", 4, side="right")
                    psF = pool(phF, f"psF{half}", 1, space="PSUM")
                    for fb in range(4):
                        for j in range(8):
                            ft = fb * 8 + j
                            ps = psF.tile([P, CH], f32, tag="big", bufs=4,
                                          name=f"h_ps{half}_{ft}")
                            steps = []
                            for c in range(KC2):
                                lh = f1_h[fb][c][:, :, j * P : (j + 1) * P]
                                ll = f1_l[fb][c][:, :, j * P : (j + 1) * P]
                                steps.append((lh, x2_8h[c][:, :, hs]))
                                steps.append((ll, x2_8h[c][:, :, hs]))
                                steps.append((lh, x2_8l[c][:, :, hs]))
                            for i, (lhsT, rhs) in enumerate(steps):
                                nc.tensor.matmul(
                                    ps, lhsT, rhs,
                                    start=(i == 0), stop=(i == len(steps) - 1),
                                    perf_mode=DR,
                                )
                            h16t = h16p.tile([P, CH], f16, tag="h16", bufs=4,
                                             name=f"h16_{half}_{ft}")
                            nc.scalar.activation(
                                h16t, ps, AF.Relu,
                                bias=fc1bc[:, ft : ft + 1], scale=1.0 / WS,
                            )
                            hh = h8h[ft // 2][:, ft % 2, :]
                            nc.gpsimd.tensor_copy(out=hh, in_=h16t)
                            nc.vector.tensor_tensor(
                                h8l[ft // 2][:, ft % 2, :], h16t, hh, OP.subtract
                            )

                with ExitStack() as phG:
                    gsb = pool(phG, f"gsb{half}", 1, side="right")
                    gmm = ExitStack()
                    psG = pool(gmm, f"psG{half}", 1, space="PSUM")
                    pss = [
                        psG.tile([P, CH], f32, tag=f"fc2_{et}",
                                 name=f"y_ps{half}_{et}")
                        for et in range(ET)
                    ]
                    for c in range(FKC2):
                        wh = wG.tile([P, 2, E], f8, tag="wf2", bufs=4,
                                     name=f"fc2h_{half}_{c}")
                        nc.sync.dma_start(wh, fc2h_d[c * P : (c + 1) * P, :, :])
                        wl = wG.tile([P, 2, E], f8, tag="wf2", bufs=4,
                                     name=f"fc2l_{half}_{c}")
                        nc.sync.dma_start(wl, fc2l_d[c * P : (c + 1) * P, :, :])
                        for et in range(ET):
                            esl = slice(et * P, (et + 1) * P)
                            steps = [
                                (wh[:, :, esl], h8h[c][:, :, :]),
                                (wl[:, :, esl], h8h[c][:, :, :]),
                                (wh[:, :, esl], h8l[c][:, :, :]),
                            ]
                            for i, (lhsT, rhs) in enumerate(steps):
                                nc.tensor.matmul(
                                    pss[et], lhsT, rhs,
                                    start=(c == 0 and i == 0),
                                    stop=(c == FKC2 - 1 and i == len(steps) - 1),
                                    perf_mode=DR,
                                )
                    r3 = [gsb.tile([P, CH], f16, name=f"r3_{half}_{et}")
                          for et in range(ET)]
                    for et in range(ET):
                        if not skipb:
                            tmp = gsb.tile([P, CH], f32, tag="f2tmp",
                                           name=f"f2t_{half}_{et}")
                            nc.vector.tensor_scalar(
                                tmp, pss[et], scalar1=1.0 / WS, op0=OP.mult,
                                scalar2=fc2bc[:, et : et + 1], op1=OP.add,
                            )
                            nc.vector.tensor_tensor(
                                r3[et], tmp, x2[et][:, hs], OP.add
                            )
                        else:
                            nc.vector.scalar_tensor_tensor(
                                r3[et], pss[et], 1.0 / WS, x2[et][:, hs],
                                op0=OP.mult, op1=OP.add,
                            )
                    gmm.close()
                    x3 = [gsb.tile([P, CH], f32, tag="x3", bufs=4,
                                   name=f"x3_{half}_{et}") for et in range(ET)]

                    def emit_out(et, hs=hs, x3=x3):
                        nc.sync.dma_start(out[et * P : (et + 1) * P, hs], x3[et])

                    layer_norm_fm(r3, gc[2], bc[2], f"ln3_{half}", x3,
                                  t0=0, t1=CH, post_et=emit_out)
                fh.close()
            wg_st.close()
            phF_st.close()

            lay.close()

        for _rep in range(reps):
            emit_layer(_rep)

    return nc


def _split8(a):
    import ml_dtypes

    e4 = ml_dtypes.float8_e4m3
    hi = a.astype(e4)
    lo = (a - hi.astype(np.float32)).astype(e4)
    return hi, lo


def _pair_layout(wT):
    """(K, N) -> (K/256, 128, 2, N) -> (K/2, 2, N): [c*128+p, i, n] = wT[c*256+i*128+p, n]"""
    K, N = wT.shape
    return np.ascontiguousarray(
        wT.reshape(K // 256, 2, P, N).transpose(0, 2, 1, 3).reshape(K // 2, 2, N)
    )


def prep_inputs(inputs):
    import ml_dtypes

    e4 = ml_dtypes.float8_e4m3
    f16c = lambda a: np.ascontiguousarray(np.asarray(a), dtype=np.float16)

    x = np.asarray(inputs["x"], np.float32)  # (S, B, E)
    z = np.asarray(inputs["z"], np.float32)  # (1, B, E)

    shared = {}
    for nm, key in (("wq", "wq"), ("wk", "wk"), ("wv", "wv"), ("wo", "wo"),
                    ("pgh", "pgh_w")):
        wT = np.asarray(inputs[key], np.float32).T * WS  # (E, E)
        hi, lo = _split8(wT)
        shared[nm + "h"] = _pair_layout(hi)
        shared[nm + "l"] = _pair_layout(lo)
    f1T = np.asarray(inputs["fc1_w"], np.float32).T * WS  # (E, F)
    hi, lo = _split8(f1T)
    shared["fc1h"] = _pair_layout(hi)
    shared["fc1l"] = _pair_layout(lo)
    f2T = np.asarray(inputs["fc2_w"], np.float32).T * WS  # (F, E)
    hi, lo = _split8(f2T)
    shared["fc2h"] = _pair_layout(hi)
    shared["fc2l"] = _pair_layout(lo)

    def colfm(v, n=ET):
        return np.asarray(v, np.float32).reshape(n, P).T  # [P, n]

    cols = np.concatenate(
        [
            colfm(np.asarray(inputs["bq"]) / np.sqrt(HD)),
            colfm(inputs["bk"]),
            colfm(inputs["bv"]),
            colfm(inputs["bo"]),
            colfm(inputs["fc2_b"]),
            colfm(inputs["ln1_g"]),
            colfm(inputs["ln2_g"]),
            colfm(inputs["ln3_g"]),
            colfm(inputs["ln1_b"]),
            colfm(inputs["ln2_b"]),
            colfm(inputs["ln3_b"]),
            colfm(inputs["fc1_b"], F1T),
        ],
        axis=1,
    )
    shared["colpack"] = np.ascontiguousarray(cols, np.float32)

    ti = np.arange(P)
    shared["cmask8"] = np.where(
        ti[None, :] >= ti[:, None], 0.0, MASKVAL
    ).astype(e4)

    skipb = all(
        not np.asarray(inputs[k]).any()
        for k in ("bv", "bo", "fc2_b", "ln1_b", "ln2_b", "ln3_b")
    )
    flags = ("skipb",) if skipb else ()

    # z projections computed on host in fp32 (input-derived, exact)
    pgzT = np.asarray(inputs["pgz_w"], np.float32).T
    pvT = np.asarray(inputs["pv_w"], np.float32).T
    zg_all = z[0] @ pgzT + np.asarray(inputs["pgz_b"], np.float32) + np.asarray(
        inputs["pgh_b"], np.float32
    )  # (B, E)
    zv_all = z[0] @ pvT + np.asarray(inputs["pv_b"], np.float32)  # (B, E)

    in_maps = []
    for b in range(B):
        xT = x[:, b, :].T  # (E, S)
        hi, lo = _split8(xT)
        m = dict(shared)
        m["x8h"] = _pair_layout(hi.astype(np.float32)).astype(e4)
        m["x8l"] = _pair_layout(lo.astype(np.float32)).astype(e4)
        m["x16"] = f16c(xT)
        zc = np.stack([colfm(zg_all[b]), colfm(zv_all[b])], axis=1)  # (P,2,ET)
        m["zcols"] = np.ascontiguousarray(zc.reshape(P, 2 * ET), np.float32)
        in_maps.append(m)
    return in_maps, flags


_NC_CACHE = {}


def get_program(reps=1, flags=("skipb",)):
    key = (reps, flags)
    if key not in _NC_CACHE:
        _NC_CACHE[key] = build_program(flags=flags, reps=reps)
    return _NC_CACHE[key]


def kernel(**inputs):
    from concourse.bass_utils import run_bass_kernel_spmd

    in_maps, flags = prep_inputs(inputs)
    nc = get_program(flags=flags)
    res = run_bass_kernel_spmd(nc, in_maps, core_ids=list(range(B)))
    return np.stack(
        [np.ascontiguousarray(res.results[b]["out"].T) for b in range(B)], axis=1
    )


# revision 55
# speedup vs baseline: 1.2106x; 1.2106x over previous
"""Trainium2 Bass kernel for nn_AutoencoderDecoderLayer (S=1024, B=8, E=1024, NH=16, F=4096).

Strategy: data-parallel over batch B=8 -> one batch element per NeuronCore.
Per core one full decoder layer over (S=1024, E=1024) tokens.

Precision plan (validated against the fp32 reference with a numpy mirror):
  - Dense matmuls run in fp8(e4m3) using DoubleRow perf mode (0.5 PE
    cycles/output column for a 256-deep contraction = 4x the fp16 rate).
  - Weights are pre-scaled by 32 on the host (so their hi/lo split channels
    stay out of e4m3's subnormal range) and split into hi + lo fp8 pairs;
    the inverse scale rides the free psum-evacuation scale.
  - q/k/v, fc1 and fc2 are fully error-compensated (act hi/lo x weight
    hi/lo, dropping only the lo*lo term); wo and pgh compensate the weight
    side only; z projections are computed on the host in fp32.
  - Attention: scores contract k as an (hi,lo) DoubleRow pair against a
    broadcast q (stride-0 slot); probs/V accumulate token-tile pairs.
    exp outputs are fp8 with a global shift (softmax-invariant).
  - Residual stream and layernorm math are fp16/fp32.

Schedule: phase A (qkv+attention) mostly PE/Act bound; phases B..G are
pipelined over token halves so the DVE/Act-heavy LN/gate work of one half
overlaps the PE-heavy fc1/fc2 of the other.  Weight DMA is streamed ahead
of use (wo/pgh during A, fc1 during B, fc2 during F).
"""

import sys

sys.path.insert(0, "/opt/trn_rl_repo")

from contextlib import ExitStack

import numpy as np

import concourse.bass as bass
import concourse.mybir as mybir
import concourse.tile as tile
from concourse.masks import make_identity
from concourse.vector_clock import ScopedClock

P = 128
S, B, E, NH, F = 1024, 8, 1024, 16, 4096
HD = E // NH            # 64
TT = S // P             # 8 token tiles
KC2 = E // 256          # 4 contraction chunk-pairs over E
FKC2 = F // 256         # 16 chunk-pairs over F
ET = E // P             # 8 feature tiles
F1T = F // P            # 32 fc1 output tiles
NCH = 2                 # token-half chunks for F/G
CH = S // NCH           # 512
NBC = 2                 # token chunks for phase B's LN/gate pipeline
BCH = S // NBC          # 512
WS = 32.0               # host weight pre-scale
ESHIFT = -4.0           # exp shift (softmax-invariant); set vs max masked score
MASKVAL = -104.0        # additive causal mask (exact in e4m3)
DENEPS = 1e-6           # guards 1/den when an entire prob row flushes to 0

f32 = mybir.dt.float32
f16 = mybir.dt.float16
f8 = mybir.dt.float8e4
f8e5 = mybir.dt.float8e5

DR = mybir.MatmulPerfMode.DoubleRow
AF = mybir.ActivationFunctionType
OP = mybir.AluOpType

_MAX_DRAIN_WAITS = 1


def _split_drain_and_barrier(self, tick_clock, wait_clock):
    """This walrus build rejects >1 sem-wait on a CTRL Drain; split the final
    tile drain's wait list across a chain of Drains on the same engine."""
    drain_inst = self.nc.sync.drain()
    wait_clock.add_sem_waits(
        drain_inst.ins, ScopedClock({None: tick_clock.global_clock})
    )
    si = drain_inst.ins.sync_info
    if si is not None and len(si.on_wait) > _MAX_DRAIN_WAITS:
        waits = list(si.on_wait)
        drain_inst.ins.sync_info = mybir.SyncInfo(
            on_wait=waits[:_MAX_DRAIN_WAITS], on_update=list(si.on_update)
        )
        rest = waits[_MAX_DRAIN_WAITS:]
        for i in range(0, len(rest), _MAX_DRAIN_WAITS):
            extra = self.nc.sync.drain()
            extra.ins.sync_info = mybir.SyncInfo(
                on_wait=rest[i : i + _MAX_DRAIN_WAITS], on_update=[]
            )
    self.nc.all_engine_barrier()
    assert self.sems is not None
    popped = self.nc._tile_sem_poison_stack.pop()
    assert popped is self._sem_poison
    self.nc.clear_and_free_semaphores(list(self.sems.allocated().values()))
    self.nc.all_engine_barrier()


tile.TileContext._drain_and_barrier = _split_drain_and_barrier


def _split_waits_in_bir(bir_bytes):
    """This walrus build accepts at most ONE sem-wait per instruction.
    Hoist extra on_wait entries onto NoOp instructions inserted just before
    the owning instruction on the same engine (waits AND together, and each
    engine executes its stream in order, so this is semantics-preserving)."""
    import json

    d = json.loads(bir_bytes)
    cnt = 0

    def fix_block(blk):
        nonlocal cnt
        insts = blk.get("instructions") or []
        out = []
        for ins in insts:
            si = ins.get("sync_info")
            if si:
                waits = si.get("on_wait") or []
                if len(waits) > 1:
                    for w in waits[:-1]:
                        cnt += 1
                        out.append(
                            {
                                "name": f"wsplit-{cnt}",
                                "opcode": "NoOp",
                                "engine": ins["engine"],
                                "ins": [],
                                "outs": [],
                                "sync_info": {"on_wait": [w], "on_update": []},
                            }
                        )
                    si["on_wait"] = waits[-1:]
            out.append(ins)
        blk["instructions"] = out
        for sub in blk.get("blocks") or []:
            fix_block(sub)

    for fn in d.get("functions", []):
        for b in fn.get("blocks", []):
            fix_block(b)
    return json.dumps(d).encode()


def _install_bir_wait_split():
    from concourse import bass2jax, bass_utils

    if getattr(bass_utils, "_orig_compile_bir_kernel", None) is None:
        bass_utils._orig_compile_bir_kernel = bass_utils.compile_bir_kernel

        def patched(bir_json, tmpdir, neff_name="file.neff"):
            return bass_utils._orig_compile_bir_kernel(
                _split_waits_in_bir(bir_json), tmpdir, neff_name=neff_name
            )

        bass_utils.compile_bir_kernel = patched
        bass2jax.compile_bir_kernel = patched


_install_bir_wait_split()


def build_program(flags=("skipb",), reps=1):
    """flags: 'skipb' present -> ln b-vectors are all-zero and bo/fc2b/bv are
    zero, so their (token-broadcast) adds can be skipped."""
    skipb = "skipb" in flags
    skipg = "skipg" in flags      # all ln gains are exactly 1.0
    p3_qkv = "q2" not in flags    # 3rd (w_hi x act_lo) pass for q/k/v
    p3_fc1 = "f12" not in flags   # 3rd pass for fc1
    p3_fc2 = "f22" not in flags   # 3rd pass for fc2
    p2_wo = "wo1" not in flags    # w-lo compensation pass for wo
    p2_pgh = "pgh1" not in flags  # w-lo compensation pass for pgh
    nc = bass.Bass("TRN2", target_bir_lowering=False, debug=False, num_devices=1)

    def din(name, shape, dt):
        return nc.dram_tensor(name, shape, dt, kind="ExternalInput").ap()

    # activations
    x8h_d = din("x8h", (KC2 * P, 2, S), f8)
    x8l_d = din("x8l", (KC2 * P, 2, S), f8)
    x16_d = din("x16", (E, S), f16)
    zcols_d = din("zcols", (P, 2 * ET), f32)  # host: [zg cols | zv cols]
    # fp8 weights (pre-scaled x32, W.T chunk-pair layout [c*128+p, i, f])
    wname = {}
    for nm in ("wq", "wk", "wv", "wo", "pgh"):
        for hl in ("h", "l"):
            wname[nm + hl] = din(nm + hl, (KC2 * P, 2, E), f8)
    fc1h_d = din("fc1h", (KC2 * P, 2, F), f8)
    fc1l_d = din("fc1l", (KC2 * P, 2, F), f8)
    fc2h_d = din("fc2h", (FKC2 * P, 2, E), f8)
    fc2l_d = din("fc2l", (FKC2 * P, 2, E), f8)
    # packed per-feature columns: 11 x [P, ET] + fc1 [P, F1T] = [P, 120]
    cols_d = din("colpack", (P, 11 * ET + F1T), f32)
    cmask_d = din("cmask8", (P, P), f8)
    out = nc.dram_tensor("out", (E, S), f16, kind="ExternalOutput").ap()

    with tile.TileContext(nc) as tc, ExitStack() as top:
        pool = lambda st, nm, bufs, **kw: st.enter_context(
            tc.tile_pool(name=nm, bufs=bufs, **kw)
        )
        const = pool(top, "const", 1, side="left")

        # ---------------- constants ----------------
        ident16 = const.tile([P, P], f16, name="ident16")
        make_identity(nc, ident16)
        ident32 = const.tile([P, P], f32, name="ident32")
        make_identity(nc, ident32)
        maskz = const.tile([P, 2, P], f8, name="maskz")  # mask slot0, zero slot1
        nc.vector.memset(maskz, 0.0)
        nc.sync.dma_start(maskz[:, 0, :], cmask_d)
        identz = const.tile([P, 2, P], f8, name="identz")
        nc.vector.memset(identz, 0.0)
        make_identity(nc, identz[:, 0, :])
        ones1 = const.tile([P, 1], f16, name="ones1")
        nc.vector.memset(ones1, 1.0)
        ones_row = const.tile([1, P], f16, name="ones_row")
        nc.vector.memset(ones_row, 1.0)
        eps_t = const.tile([P, 1], f32, name="eps_t")
        nc.vector.memset(eps_t, 1e-5)
        eshift_t = const.tile([P, 1], f32, name="eshift_t")
        nc.vector.memset(eshift_t, ESHIFT)

        colpack = const.tile([P, 11 * ET + F1T], f32, name="colpack_sb")
        nc.sync.dma_start(colpack, cols_d)
        _c = [colpack[:, i * ET : (i + 1) * ET] for i in range(11)]
        bqc, bkc, bvc, boc, fc2bc = _c[0], _c[1], _c[2], _c[3], _c[4]
        gc = _c[5:8]
        bc = _c[8:11]
        fc1bc = colpack[:, 11 * ET : 11 * ET + F1T]

        zgv = const.tile([P, 2, ET], f32, name="zgv_sb")
        nc.sync.dma_start(
            zgv, zcols_d.rearrange("p (j e) -> p j e", j=2)
        )

        def emit_layer(rep):
            lay = ExitStack()
            stat_sb = pool(lay, "stat_sb", 2, side="left")  # small stats
            keep = pool(lay, "keep", 1, side="left")        # x2 + x2_8 [B..G]

            # ---------- LN helpers (feature-major, trickled stats) ----------
            # ln_begin allocates the stats psum; ln_et (called per feature
            # tile, in order) emits sq + the two 1-col stats matmuls so they
            # overlap the producer; ln_finish does mu/rstd, the token-row
            # broadcast and the normalize.
            def ln_begin(nm, nt, ln_pools):
                ps_st, sq_p = ln_pools
                ntb = nt // P
                xq = ps_st.tile([P, 2, ntb], f32, tag="xq", bufs=2,
                                name=f"xq_{nm}")
                return {"nm": nm, "nt": nt, "ntb": ntb, "xq": xq,
                        "ps": ps_st, "sqp": sq_p}

            def ln_et(st, rslice, et):
                nt, ntb = st["nt"], st["ntb"]
                sq = st["sqp"].tile([P, nt], f16, tag="sq", bufs=4,
                                    name=f"sq_{st['nm']}_{et}")
                if et % 2 == 0:
                    nc.vector.tensor_tensor(sq, rslice, rslice, OP.mult)
                else:
                    nc.scalar.activation(sq, rslice, AF.Square)
                xs = st["xq"][:, 0, :]
                qs = st["xq"][:, 1, :]
                for tb in range(ntb):
                    nc.tensor.matmul(
                        xs[:, tb : tb + 1], rslice[:, tb * P : (tb + 1) * P],
                        ones1, start=(et == 0 and tb == 0), stop=False,
                        skip_group_check=True,
                    )
                    nc.tensor.matmul(
                        qs[:, tb : tb + 1], sq[:, tb * P : (tb + 1) * P],
                        ones1, start=False,
                        stop=(et == ET - 1 and tb == ntb - 1),
                        skip_group_check=True,
                    )

            def ln_finish(st, rtiles, g_col, b_col, out_tiles, t0=0, ot0=None,
                          post_et=None, pool_ets=(2, 5)):
                nm, nt, ntb = st["nm"], st["nt"], st["ntb"]
                if ot0 is None:
                    ot0 = t0
                ps_st, sq_p = st["ps"], st["sqp"]
                xs = st["xq"][:, 0, :]
                qs = st["xq"][:, 1, :]
                mu = stat_sb.tile([P, ntb], f32, tag="mu", name=f"mu_{nm}")
                nc.vector.tensor_scalar(
                    mu, xs, scalar1=1.0 / E, scalar2=None, op0=OP.mult
                )
                msq = stat_sb.tile([P, ntb], f32, tag="msq", name=f"msq_{nm}")
                nc.vector.tensor_tensor(msq, mu, mu, OP.mult)
                var = stat_sb.tile([P, ntb], f32, tag="var", name=f"var_{nm}")
                nc.vector.scalar_tensor_tensor(
                    var, qs, 1.0 / E, msq, op0=OP.mult, op1=OP.subtract
                )
                sd = stat_sb.tile([P, ntb], f32, tag="sd", name=f"sd_{nm}")
                nc.scalar.activation(sd, var, AF.Sqrt, bias=eps_t, scale=1.0)
                st16 = stat_sb.tile([P, 2, ntb], f16, tag="st16", name=f"st16_{nm}")
                with nc.allow_low_precision(reason="fp16 rstd is ample for LN"):
                    nc.vector.reciprocal(st16[:, 0, :], sd)
                nc.vector.scalar_tensor_tensor(
                    st16[:, 1, :], mu, -1.0, st16[:, 0, :], op0=OP.mult, op1=OP.mult
                )
                pr = ps_st.tile([33, nt], f16, tag="str", bufs=1, name=f"str_{nm}")
                for b in range(ntb):
                    nc.tensor.transpose(
                        pr[0:1, b * P : (b + 1) * P], st16[:, 0, b : b + 1], ident16
                    )
                    nc.tensor.transpose(
                        pr[32:33, b * P : (b + 1) * P], st16[:, 1, b : b + 1], ident16
                    )
                row_r = stat_sb.tile([1, nt], f16, tag="rowr", name=f"rowr_{nm}")
                nc.scalar.activation(row_r, pr[0:1, :], AF.Copy)
                row_n = stat_sb.tile([1, nt], f16, tag="rown", name=f"rown_{nm}")
                nc.scalar.activation(row_n, pr[32:33, :], AF.Copy)
                rstd_bc = stat_sb.tile([P, nt], f16, tag="rbc", name=f"rbc_{nm}")
                nmur_bc = stat_sb.tile([P, nt], f16, tag="nbc", name=f"nbc_{nm}")
                for j, (row, bcst) in enumerate(((row_r, rstd_bc), (row_n, nmur_bc))):
                    for halfn in range(0, nt, 512):
                        nn = min(512, nt - halfn)
                        pbc = ps_st.tile([P, 512], f32, tag="pbc", bufs=2,
                                         name=f"pbc_{nm}_{j}_{halfn}")
                        nc.tensor.matmul(
                            pbc[:, 0:nn], ones_row,
                            row[:, halfn : halfn + nn],
                            start=True, stop=True,
                        )
                        if j == 0:
                            nc.vector.tensor_copy(
                                out=bcst[:, halfn : halfn + nn], in_=pbc[:, 0:nn]
                            )
                        else:
                            nc.scalar.activation(
                                bcst[:, halfn : halfn + nn], pbc[:, 0:nn], AF.Copy
                            )
                for et in range(ET):
                    osl = slice(ot0, ot0 + nt)
                    t = sq_p.tile([P, nt], f16, tag="tn", name=f"tn_{nm}_{et}")
                    if skipg:
                        eng = nc.gpsimd if et in pool_ets else nc.vector
                        eng.tensor_tensor(
                            t, rtiles[et][:, t0 : t0 + nt], rstd_bc, OP.mult
                        )
                        eng.tensor_tensor(
                            out_tiles[et][:, osl], t, nmur_bc, OP.add
                        )
                    else:
                        nc.vector.scalar_tensor_tensor(
                            t, rtiles[et][:, t0 : t0 + nt], g_col[:, et : et + 1],
                            rstd_bc, op0=OP.mult, op1=OP.mult,
                        )
                        nc.vector.scalar_tensor_tensor(
                            out_tiles[et][:, osl], nmur_bc,
                            g_col[:, et : et + 1], t, op0=OP.mult, op1=OP.add,
                        )
                    if not skipb:
                        nc.vector.tensor_scalar(
                            out_tiles[et][:, osl], out_tiles[et][:, osl],
                            scalar1=b_col[:, et : et + 1], scalar2=None,
                            op0=OP.add,
                        )
                    if post_et is not None:
                        post_et(et)

            # attention outputs + wo/pgh weights (left side; freed at layer end)
            attw_w_st = ExitStack()
            attw = pool(attw_w_st, "attw", 1, side="left")
            attw_t_st = ExitStack()
            attwt = pool(attw_t_st, "attwt", 1, side="left")
            attnT8 = [
                attwt.tile([P, 2, S], f8, name=f"attnT8_{c}") for c in range(KC2)
            ]
            wo_h, wo_l, pgh_h, pgh_l = [], [], [], []
            for c in range(KC2):
                for nm, lst in (
                    ("woh", wo_h), ("wol", wo_l), ("pghh", pgh_h), ("pghl", pgh_l)
                ):
                    lst.append(attw.tile([P, 2, E], f8, name=f"{nm}_{c}"))

            # ---------- Phase A: qkv + attention, interleaved per f-tile -----
            with ExitStack() as phA:
                xin = pool(phA, "xin", 1, side="right")
                wq_pool = pool(phA, "wq_pool", 1, side="right")

                # x first (first matmul needs it), then qkv weights, then the
                # wo/pgh prefetch into the longer-lived attw pool.
                x8h = [xin.tile([P, 2, S], f8, name=f"x8h_{c}") for c in range(KC2)]
                x8l = [xin.tile([P, 2, S], f8, name=f"x8l_{c}") for c in range(KC2)]
                qkv_w = {
                    nm: [wq_pool.tile([P, 2, E], f8, name=f"{nm}_{c}")
                         for c in range(KC2)]
                    for nm in ("wqh", "wql", "wkh", "wkl", "wvh", "wvl")
                }
                # stream in first-use order: per chunk x8h/x8l + wq, then wk, wv
                for c in range(KC2):
                    nc.sync.dma_start(x8h[c], x8h_d[c * P : (c + 1) * P, :, :])
                    nc.sync.dma_start(x8l[c], x8l_d[c * P : (c + 1) * P, :, :])
                    nc.sync.dma_start(
                        qkv_w["wqh"][c], wname["wqh"][c * P : (c + 1) * P, :, :]
                    )
                    nc.sync.dma_start(
                        qkv_w["wql"][c], wname["wql"][c * P : (c + 1) * P, :, :]
                    )
                for nm in ("wkh", "wkl", "wvh", "wvl"):
                    for c in range(KC2):
                        nc.sync.dma_start(
                            qkv_w[nm][c], wname[nm][c * P : (c + 1) * P, :, :]
                        )
                for c in range(KC2):
                    for nm, lst in (
                        ("woh", wo_h), ("wol", wo_l), ("pghh", pgh_h), ("pghl", pgh_l)
                    ):
                        nc.sync.dma_start(
                            lst[c], wname[nm][c * P : (c + 1) * P, :, :]
                        )

                asb = pool(phA, "asb", 1, side="right")
                expp = pool(phA, "expp", 12, side="right")
                a16p = pool(phA, "a16p", 3, side="right")
                psA = pool(phA, "psA", 1, space="PSUM")

                q8 = [
                    asb.tile([P, S], f8, tag="q8", bufs=3, name=f"q8_{et}")
                    for et in range(ET)
                ]
                k8 = [
                    asb.tile([P, 2, S], f8, tag="k8", bufs=3, name=f"k8_{et}")
                    for et in range(ET)
                ]
                v1p = [
                    asb.tile([P, 2, NH, HD + 1], f8, name=f"v1p_{c}")
                    for c in range(KC2)
                ]
                for c in range(KC2):
                    nc.vector.memset(v1p[c][:, :, :, HD : HD + 1], 1.0)

                def ps_half(nm):
                    return psA.tile([P, 512], f32, tag="pA", bufs=2, name=nm)

                def ps_sc(nm):
                    return psA.tile([P, 2, 512], f32, tag="sc", bufs=2, name=nm)

                ep = {}  # h -> list of 4 pair tiles (ring keeps ~3 heads)

                def proj_fm(et, whi, wlo, nm, evac):
                    for tb in range(2):
                        rs = slice(tb * 512, (tb + 1) * 512)
                        ps = ps_half(f"{nm}_ps{et}_{tb}")
                        steps = []
                        for c in range(KC2):
                            lh = whi[c][:, :, et * P : (et + 1) * P]
                            ll = wlo[c][:, :, et * P : (et + 1) * P]
                            steps.append((lh, x8h[c][:, :, rs]))
                            steps.append((ll, x8h[c][:, :, rs]))
                            if p3_qkv:
                                steps.append((lh, x8l[c][:, :, rs]))
                        for i, (lhsT, rhs) in enumerate(steps):
                            nc.tensor.matmul(
                                ps, lhsT, rhs,
                                start=(i == 0), stop=(i == len(steps) - 1),
                                perf_mode=DR,
                            )
                        evac(ps, rs)

                def scores_head(h):
                    et = h // 2
                    r0 = (h % 2) * HD
                    # per-chunk widths follow the causal wedge: tile c only
                    # holds q columns >= 256*c (col j <-> q token 256c + j)
                    eps_tiles = [
                        expp.tile([P, 2, S - 256 * c], f8e5 if c == 0 else f8,
                                  tag=f"ep{c}5" if c == 0 else f"ep{c}",
                                  bufs=5,
                                  name=f"ep{h}_{c}")
                        for c in range(KC2)
                    ]
                    ep[h] = eps_tiles
                    for tjt in range(TT):
                        base = tjt * P
                        ncols = S - base
                        lhsT = k8[et][r0 : r0 + HD, :, base : base + P]
                        ps = ps_sc(f"sc{h}_{tjt}")
                        psf = ps.rearrange("p a b -> p (a b)")
                        off = base
                        while off < S:
                            n = min(512 - ((off - base) % 512), S - off)
                            rhs = q8[et][
                                r0 : r0 + HD, None, off : off + n
                            ].to_broadcast([HD, 2, n])
                            nc.tensor.matmul(
                                psf[:, off - base : off - base + n], lhsT, rhs,
                                start=True, stop=(off != base), perf_mode=DR,
                            )
                            if off == base:
                                nc.tensor.matmul(
                                    psf[:, 0:P], identz, maskz,
                                    start=False, stop=True, perf_mode=DR,
                                )
                            off += n
                        q0 = 256 * (tjt // 2)
                        nc.scalar.activation(
                            eps_tiles[tjt // 2][:, tjt % 2, base - q0 : S - q0],
                            psf[:, 0:ncols], AF.Exp, bias=eshift_t, scale=1.0,
                        )

                def av_pair(hp):
                    """probs @ V + evac + transpose for heads 2hp, 2hp+1."""
                    raw = a16p.tile([P, 2, TT, HD + 1], f16, tag="a16",
                                    name=f"a16_{hp}")
                    for tit in range(TT):
                        pav = psA.tile(
                            [P, 2, HD + 1], f32, tag="pav", bufs=1,
                            name=f"pav{hp}_{tit}",
                        )
                        for hh in range(2):
                            h = hp * 2 + hh
                            ept = ep[h]
                            npair = tit // 2
                            for c in range(npair):
                                q0 = 256 * c
                                nc.tensor.matmul(
                                    pav[:, hh, :],
                                    ept[c][:, :, tit * P - q0 : (tit + 1) * P - q0],
                                    v1p[c][:, :, h, :],
                                    start=(c == 0), stop=False, perf_mode=DR,
                                )
                            for tj in range(2 * npair, tit + 1):
                                q0 = 256 * (tj // 2)
                                nc.tensor.matmul(
                                    pav[:, hh, :],
                                    ept[tj // 2][
                                        :, tj % 2, tit * P - q0 : (tit + 1) * P - q0
                                    ],
                                    v1p[tj // 2][:, tj % 2, h, :],
                                    start=(tj == 0), stop=(tj == tit),
                                )
                        # evacuate raw numerator+den quickly; divide later in
                        # one batched op (keeps the pav psum ping-pong short)
                        nc.vector.tensor_copy(out=raw[:, :, tit, :], in_=pav)
                    den = stat_sb.tile([P, 2, TT], f32, tag="den", name=f"den{hp}")
                    nc.vector.tensor_scalar(
                        den, raw[:, :, :, HD], scalar1=DENEPS, scalar2=None,
                        op0=OP.add,
                    )
                    rc = stat_sb.tile([P, 2, TT], f16, tag="rc", name=f"rc{hp}")
                    with nc.allow_low_precision(reason="fp16 1/den ample for probs"):
                        nc.vector.reciprocal(rc, den)
                    # tt-major so each transpose source [:, tt, :, :] is a
                    # single mergeable 128-wide free dim (walrus requirement)
                    a16x = a16p.tile([P, TT, 2, HD], f16, tag="a16x",
                                     name=f"a16x_{hp}")
                    nc.vector.tensor_tensor(
                        a16x.rearrange("p t h d -> p h t d"),
                        raw[:, :, :, 0:HD],
                        rc[:, :, :, None].to_broadcast([P, 2, TT, HD]), OP.mult,
                    )
                    # transpose: [tok, (2 heads, d)] per tt -> attnT8 FM slot
                    for tt in range(TT):
                        pt = psA.tile([P, P], f16, tag="tr", bufs=1, name=f"trA{hp}_{tt}")
                        nc.tensor.transpose(
                            pt,
                            a16x[:, tt, :, :],
                            ident16,
                        )
                        dstT = attnT8[hp // 2][:, hp % 2, tt * P : (tt + 1) * P]
                        nc.vector.tensor_copy(out=dstT, in_=pt)

                for et in range(ET):
                    def evac_q(ps, rs, et=et):
                        nc.vector.tensor_scalar(
                            q8[et][:, rs], ps, 1.0 / (WS * 8.0),
                            bqc[:, et : et + 1], op0=OP.mult, op1=OP.add,
                        )
                    proj_fm(et, qkv_w["wqh"], qkv_w["wql"], "q", evac_q)

                    def evac_k(ps, rs, et=et):
                        nc.vector.tensor_scalar(
                            k8[et][:, 0, rs], ps, 1.0 / WS,
                            bkc[:, et : et + 1], op0=OP.mult, op1=OP.add,
                        )
                        nc.vector.scalar_tensor_tensor(
                            k8[et][:, 1, rs], ps, 1.0 / WS, k8[et][:, 0, rs],
                            op0=OP.mult, op1=OP.subtract,
                        )
                    proj_fm(et, qkv_w["wkh"], qkv_w["wkl"], "k", evac_k)
                    if et % 4 == 0:
                        c4 = et // 4
                        for tt in range(TT):
                            psv = ps_half(f"v_ps{c4}_{tt}")
                            steps = []
                            for c in range(KC2):
                                lh = x8h[c][:, :, tt * P : (tt + 1) * P]
                                ll = x8l[c][:, :, tt * P : (tt + 1) * P]
                                wh = qkv_w["wvh"][c][:, :, c4 * 512 : (c4 + 1) * 512]
                                wl = qkv_w["wvl"][c][:, :, c4 * 512 : (c4 + 1) * 512]
                                steps += [(lh, wh), (ll, wh)]
                                if p3_qkv:
                                    steps.append((lh, wl))
                            for i, (lhsT, rhs) in enumerate(steps):
                                nc.tensor.matmul(
                                    psv, lhsT, rhs,
                                    start=(i == 0), stop=(i == len(steps) - 1),
                                    perf_mode=DR,
                                )
                            nc.vector.tensor_scalar(
                                v1p[tt // 2][:, tt % 2, c4 * 8 : (c4 + 1) * 8, 0:HD],
                                psv.rearrange("p (h d) -> p h d", d=HD),
                                1.0 / WS, bvc[:, et : et + 1],
                                op0=OP.mult, op1=OP.add,
                            )
                    scores_head(2 * et)
                    scores_head(2 * et + 1)
                    if et >= 1:
                        av_pair(et - 1)
                av_pair(ET - 1)

            # ---------- Phases B..G: token-half pipeline ---------------------
            x2 = [keep.tile([P, S], f16, name=f"x2_{et}") for et in range(ET)]
            x2_8h = [keep.tile([P, 2, S], f8, name=f"x2_8h_{c}") for c in range(KC2)]
            x2_8l = [keep.tile([P, 2, S], f8, name=f"x2_8l_{c}") for c in range(KC2)]

            # fc1 weights resident [B..F]; DMA streams during phase B.
            # fc1 weights stream through a 2-fb ring; fb0/fb1 go out on the
            # SP queue (arrive during B), fb2/fb3 on the Pool DGE queue so
            # their ring-slot waits don't block the fc2 stream behind them.
            phF_st = ExitStack()
            wFr = pool(phF_st, "wFr", 16, side="right")
            x16_st = ExitStack()
            x16p = pool(x16_st, "x16p", 1, side="right")
            x16sb = [x16p.tile([P, S], f16, name=f"x16_{et}") for et in range(ET)]
            for et in range(ET):
                nc.sync.dma_start(x16sb[et], x16_d[et * P : (et + 1) * P, :])
            f1_h = [[None] * KC2 for _ in range(4)]
            f1_l = [[None] * KC2 for _ in range(4)]
            for fb in range(4):
                dma = nc.sync.dma_start if fb < 2 else nc.gpsimd.dma_start
                for c in range(KC2):
                    th = wFr.tile([P, 2, 8 * P], f8, tag="wf1", bufs=16,
                                  name=f"f1h_{fb}_{c}")
                    dma(th, fc1h_d[c * P : (c + 1) * P, :, fb * 1024 : (fb + 1) * 1024])
                    f1_h[fb][c] = th
                    tl = wFr.tile([P, 2, 8 * P], f8, tag="wf1", bufs=16,
                                  name=f"f1l_{fb}_{c}")
                    dma(tl, fc1l_d[c * P : (c + 1) * P, :, fb * 1024 : (fb + 1) * 1024])
                    f1_l[fb][c] = tl

            # ----- phase B: o-proj (both halves hoisted), then per-half
            #       LN1 -> gate -> LN2.  The hoist keeps the PE fed with the
            #       other half's matmuls while DVE/Act chew on layernorms.
            with ExitStack() as phB:
                psB = pool(phB, "psB", 1, space="PSUM")
                bsb = pool(phB, "bsb", 1, side="right")
                ps_ln = pool(phB, "ps_lnB", 2, space="PSUM")
                sq_ln = pool(phB, "sq_lnB", 2, side="right")
                lnp = (ps_ln, sq_ln)
                r1 = [[None] * ET for _ in range(NBC)]
                ln1st = [ln_begin(f"ln1_{h}", BCH, lnp) for h in range(NBC)]
                for half in range(NBC):
                    hs = slice(half * BCH, (half + 1) * BCH)
                    for et in range(ET):
                        ps = psB.tile([P, BCH], f32, tag="big", bufs=3,
                                      name=f"o_ps{half}_{et}")
                        steps = []
                        for c in range(KC2):
                            steps.append((wo_h[c][:, :, et * P : (et + 1) * P],
                                          attnT8[c][:, :, hs]))
                            if p2_wo:
                                steps.append((wo_l[c][:, :, et * P : (et + 1) * P],
                                              attnT8[c][:, :, hs]))
                        for i, (lhsT, rhs) in enumerate(steps):
                            nc.tensor.matmul(
                                ps, lhsT, rhs,
                                start=(i == 0), stop=(i == len(steps) - 1),
                                perf_mode=DR,
                            )
                        # r1 overwrites the x16 residual tile in place
                        x16v = x16sb[et][:, hs]
                        r1[half][et] = x16v
                        if not skipb:
                            tmp = bsb.tile([P, BCH], f32, tag="botmp", bufs=4,
                                           name=f"bo_{half}_{et}")
                            nc.vector.tensor_scalar(
                                tmp, ps, scalar1=1.0 / WS, op0=OP.mult,
                                scalar2=boc[:, et : et + 1], op1=OP.add,
                            )
                            nc.vector.tensor_tensor(x16v, tmp, x16v, OP.add)
                        elif et % 2 == 0:
                            nc.vector.scalar_tensor_tensor(
                                x16v, ps, 1.0 / WS, x16v, op0=OP.mult, op1=OP.add
                            )
                        else:
                            # balance: Act evac + gpsimd residual add
                            tmp = bsb.tile([P, BCH], f16, tag="botmp16", bufs=2,
                                           name=f"bo_{half}_{et}")
                            nc.scalar.activation(tmp, ps, AF.Copy, scale=1.0 / WS)
                            nc.gpsimd.tensor_tensor(x16v, tmp, x16v, OP.add)
                        if et >= 1:
                            ln_et(ln1st[half], r1[half][et - 1], et - 1)
                    ln_et(ln1st[half], r1[half][ET - 1], ET - 1)
                attw_t_st.close()

                # stage-interleaved emission across halves: half1's
                # independent ops fill half0's dependency stalls
                x1 = [[bsb.tile([P, BCH], f16, tag="x1", bufs=16,
                                name=f"x1_{half}_{et}") for et in range(ET)]
                      for half in range(NBC)]
                x1_8 = [[bsb.tile([P, 2, BCH], f8, tag="x1_8", bufs=8,
                                  name=f"x1_8_{half}_{c}") for c in range(KC2)]
                        for half in range(NBC)]

                for half in range(NBC):
                    ln_finish(ln1st[half], r1[half], gc[0], bc[0], x1[half])
                for half in range(NBC):
                    for et in range(ET):
                        dst = x1_8[half][et // 2][:, et % 2, :]
                        if et % 2 == 0:
                            nc.scalar.activation(dst, x1[half][et], AF.Copy)
                        else:
                            nc.vector.tensor_copy(out=dst, in_=x1[half][et])

                for half in range(NBC):
                    st2 = ln_begin(f"ln2_{half}", BCH, lnp)
                    for et in range(ET):
                        ps = psB.tile([P, BCH], f32, tag="big", bufs=3,
                                      name=f"g_ps{half}_{et}")
                        steps = []
                        for c in range(KC2):
                            steps.append((pgh_h[c][:, :, et * P : (et + 1) * P],
                                          x1_8[half][c][:, :, :]))
                            if p2_pgh:
                                steps.append((pgh_l[c][:, :, et * P : (et + 1) * P],
                                              x1_8[half][c][:, :, :]))
                        for i, (lhsT, rhs) in enumerate(steps):
                            nc.tensor.matmul(
                                ps, lhsT, rhs,
                                start=(i == 0), stop=(i == len(steps) - 1),
                                perf_mode=DR,
                            )
                        gate = bsb.tile([P, BCH], f16, tag="gate", bufs=4,
                                        name=f"gate_{half}_{et}")
                        nc.scalar.activation(
                            gate, ps, AF.Sigmoid,
                            bias=zgv[:, 0, et : et + 1], scale=1.0 / WS,
                        )
                        # r2 overwrites x1 in place: r2 = gate*zv + x1
                        nc.vector.scalar_tensor_tensor(
                            x1[half][et], gate, zgv[:, 1, et : et + 1],
                            x1[half][et], op0=OP.mult, op1=OP.add,
                        )
                        if et >= 1:
                            ln_et(st2, x1[half][et - 1], et - 1)
                    ln_et(st2, x1[half][ET - 1], ET - 1)
                    ln_finish(st2, x1[half], gc[1], bc[1], x2, ot0=half * BCH)
                for half in range(NBC):
                    hs = slice(half * BCH, (half + 1) * BCH)
                    for et in range(ET):
                        hslot = x2_8h[et // 2][:, et % 2, hs]
                        if et % 2 == 0:
                            nc.scalar.activation(hslot, x2[et][:, hs], AF.Copy)
                        else:
                            nc.gpsimd.tensor_copy(out=hslot, in_=x2[et][:, hs])
                        enl = nc.vector if et % 2 == 0 else nc.gpsimd
                        enl.tensor_tensor(
                            x2_8l[et // 2][:, et % 2, hs], x2[et][:, hs], hslot,
                            OP.subtract,
                        )
            x16_st.close()
            attw_w_st.close()

            # ----- phase F (fc1, fb-major) then G (fc2 et-major + LN3) -----
            # fc2 weights resident (streamed once during B/F); h8 full-S so
            # G(0)'s LN3 tail overlaps G(1)'s matmuls via small psum rings.
            wg_st = ExitStack()
            wG = pool(wg_st, "wG", 1, side="left")
            wg_h, wg_l = [], []
            for c in range(FKC2):
                wh = wG.tile([P, 2, E], f8, name=f"fc2h_{c}")
                nc.sync.dma_start(wh, fc2h_d[c * P : (c + 1) * P, :, :])
                wg_h.append(wh)
                wl = wG.tile([P, 2, E], f8, name=f"fc2l_{c}")
                nc.sync.dma_start(wl, fc2l_d[c * P : (c + 1) * P, :, :])
                wg_l.append(wl)
            h8_st = ExitStack()
            h8p = pool(h8_st, "h8p", 1, side="left")
            h8h = [h8p.tile([P, 2, S], f8, name=f"h8h_{c}") for c in range(FKC2)]
            h8l = [h8p.tile([P, 2, S], f8, name=f"h8l_{c}") for c in range(FKC2)]

            with ExitStack() as phF:
                h16p = pool(phF, "h16p", 4, side="right")
                psF = pool(phF, "psF", 1, space="PSUM")
                for fb in range(4):
                    for half in range(NCH):
                        hs = slice(half * CH, (half + 1) * CH)
                        for j in range(8):
                            ft = fb * 8 + j
                            ps = psF.tile([P, CH], f32, tag="big", bufs=4,
                                          name=f"h_ps{half}_{ft}")
                            steps = []
                            for c in range(KC2):
                                lh = f1_h[fb][c][:, :, j * P : (j + 1) * P]
                                ll = f1_l[fb][c][:, :, j * P : (j + 1) * P]
                                steps.append((lh, x2_8h[c][:, :, hs]))
                                steps.append((ll, x2_8h[c][:, :, hs]))
                                if p3_fc1:
                                    steps.append((lh, x2_8l[c][:, :, hs]))
                            for i, (lhsT, rhs) in enumerate(steps):
                                nc.tensor.matmul(
                                    ps, lhsT, rhs,
                                    start=(i == 0), stop=(i == len(steps) - 1),
                                    perf_mode=DR,
                                )
                            h16t = h16p.tile([P, CH], f16, tag="h16", bufs=4,
                                             name=f"h16_{half}_{ft}")
                            nc.scalar.activation(
                                h16t, ps, AF.Relu,
                                bias=fc1bc[:, ft : ft + 1], scale=1.0 / WS,
                            )
                            hh = h8h[ft // 2][:, ft % 2, hs]
                            # spread hi-copy / lo-sub across Act/DVE/Pool
                            if ft % 3 == 0:
                                nc.scalar.activation(hh, h16t, AF.Copy)
                            elif ft % 3 == 1:
                                nc.vector.tensor_copy(out=hh, in_=h16t)
                            else:
                                nc.gpsimd.tensor_copy(out=hh, in_=h16t)
                            seng = nc.vector if ft % 2 == 0 else nc.gpsimd
                            seng.tensor_tensor(
                                h8l[ft // 2][:, ft % 2, hs], h16t, hh, OP.subtract
                            )
            phF_st.close()

            with ExitStack() as phG:
                gsb = pool(phG, "gsb", 1, side="right")
                psG = pool(phG, "psG", 1, space="PSUM")
                ps_ln3 = pool(phG, "ps_ln3", 1, space="PSUM")
                sq_ln3 = pool(phG, "sq_ln3", 2, side="right")
                lnp3 = (ps_ln3, sq_ln3)
                for half in range(NCH):
                    hs = slice(half * CH, (half + 1) * CH)
                    r3 = [gsb.tile([P, CH], f16, name=f"r3_{half}_{et}")
                          for et in range(ET)]
                    st3 = ln_begin(f"ln3_{half}", CH, lnp3)
                    for et in range(ET):
                        ps = psG.tile([P, CH], f32, tag="big", bufs=3,
                                      name=f"y_ps{half}_{et}")
                        esl = slice(et * P, (et + 1) * P)
                        steps = []
                        for c in range(FKC2):
                            steps.append((wg_h[c][:, :, esl], h8h[c][:, :, hs]))
                            steps.append((wg_l[c][:, :, esl], h8h[c][:, :, hs]))
                            if p3_fc2:
                                steps.append((wg_h[c][:, :, esl], h8l[c][:, :, hs]))
                        for i, (lhsT, rhs) in enumerate(steps):
                            nc.tensor.matmul(
                                ps, lhsT, rhs,
                                start=(i == 0), stop=(i == len(steps) - 1),
                                perf_mode=DR,
                            )
                        if not skipb:
                            tmp = gsb.tile([P, CH], f32, tag="f2tmp", bufs=2,
                                           name=f"f2t_{half}_{et}")
                            nc.vector.tensor_scalar(
                                tmp, ps, scalar1=1.0 / WS, op0=OP.mult,
                                scalar2=fc2bc[:, et : et + 1], op1=OP.add,
                            )
                            nc.vector.tensor_tensor(
                                r3[et], tmp, x2[et][:, hs], OP.add
                            )
                        elif et % 2 == 0:
                            nc.vector.scalar_tensor_tensor(
                                r3[et], ps, 1.0 / WS, x2[et][:, hs],
                                op0=OP.mult, op1=OP.add,
                            )
                        else:
                            tmp = gsb.tile([P, CH], f16, tag="f2t16", bufs=2,
                                           name=f"f2t_{half}_{et}")
                            nc.scalar.activation(tmp, ps, AF.Copy,
                                                 scale=1.0 / WS)
                            nc.gpsimd.tensor_tensor(r3[et], tmp, x2[et][:, hs],
                                                    OP.add)
                        if et >= 1:
                            ln_et(st3, r3[et - 1], et - 1)
                    ln_et(st3, r3[ET - 1], ET - 1)
                    x3 = [gsb.tile([P, CH], f16, tag="x3", bufs=4,
                                   name=f"x3_{half}_{et}") for et in range(ET)]

                    def emit_out(et, hs=hs, x3=x3):
                        nc.sync.dma_start(out[et * P : (et + 1) * P, hs], x3[et])

                    ln_finish(st3, r3, gc[2], bc[2], x3, post_et=emit_out,
                              pool_ets=())
            h8_st.close()
            wg_st.close()

            lay.close()

        for _rep in range(reps):
            emit_layer(_rep)

    return nc


def _split8(a):
    import ml_dtypes

    e4 = ml_dtypes.float8_e4m3
    hi = a.astype(e4)
    lo = (a - hi.astype(np.float32)).astype(e4)
    return hi, lo


def _pair_layout(wT):
    """(K, N) -> (K/256, 128, 2, N) -> (K/2, 2, N): [c*128+p, i, n] = wT[c*256+i*128+p, n]"""
    K, N = wT.shape
    return np.ascontiguousarray(
        wT.reshape(K // 256, 2, P, N).transpose(0, 2, 1, 3).reshape(K // 2, 2, N)
    )


def prep_inputs(inputs):
    import ml_dtypes

    e4 = ml_dtypes.float8_e4m3
    f16c = lambda a: np.ascontiguousarray(np.asarray(a), dtype=np.float16)

    x = np.asarray(inputs["x"], np.float32)  # (S, B, E)
    z = np.asarray(inputs["z"], np.float32)  # (1, B, E)

    shared = {}
    for nm, key in (("wq", "wq"), ("wk", "wk"), ("wv", "wv"), ("wo", "wo"),
                    ("pgh", "pgh_w")):
        wT = np.asarray(inputs[key], np.float32).T * WS  # (E, E)
        hi, lo = _split8(wT)
        shared[nm + "h"] = _pair_layout(hi)
        shared[nm + "l"] = _pair_layout(lo)
    f1T = np.asarray(inputs["fc1_w"], np.float32).T * WS  # (E, F)
    hi, lo = _split8(f1T)
    shared["fc1h"] = _pair_layout(hi)
    shared["fc1l"] = _pair_layout(lo)
    f2T = np.asarray(inputs["fc2_w"], np.float32).T * WS  # (F, E)
    hi, lo = _split8(f2T)
    shared["fc2h"] = _pair_layout(hi)
    shared["fc2l"] = _pair_layout(lo)

    def colfm(v, n=ET):
        return np.asarray(v, np.float32).reshape(n, P).T  # [P, n]

    cols = np.concatenate(
        [
            colfm(np.asarray(inputs["bq"]) / np.sqrt(HD)),
            colfm(inputs["bk"]),
            colfm(inputs["bv"]),
            colfm(inputs["bo"]),
            colfm(inputs["fc2_b"]),
            colfm(inputs["ln1_g"]),
            colfm(inputs["ln2_g"]),
            colfm(inputs["ln3_g"]),
            colfm(inputs["ln1_b"]),
            colfm(inputs["ln2_b"]),
            colfm(inputs["ln3_b"]),
            colfm(inputs["fc1_b"], F1T),
        ],
        axis=1,
    )
    shared["colpack"] = np.ascontiguousarray(cols, np.float32)

    ti = np.arange(P)
    shared["cmask8"] = np.where(
        ti[None, :] >= ti[:, None], 0.0, MASKVAL
    ).astype(e4)

    skipb = all(
        not np.asarray(inputs[k]).any()
        for k in ("bv", "bo", "fc2_b", "ln1_b", "ln2_b", "ln3_b")
    )
    skipg = all(
        bool(np.all(np.asarray(inputs[k]) == 1.0))
        for k in ("ln1_g", "ln2_g", "ln3_g")
    )
    flags = (("skipb",) if skipb else ()) + (("skipg",) if skipg else ())

    # z projections computed on host in fp32 (input-derived, exact)
    pgzT = np.asarray(inputs["pgz_w"], np.float32).T
    pvT = np.asarray(inputs["pv_w"], np.float32).T
    zg_all = z[0] @ pgzT + np.asarray(inputs["pgz_b"], np.float32) + np.asarray(
        inputs["pgh_b"], np.float32
    )  # (B, E)
    zv_all = z[0] @ pvT + np.asarray(inputs["pv_b"], np.float32)  # (B, E)

    in_maps = []
    for b in range(B):
        xT = x[:, b, :].T  # (E, S)
        hi, lo = _split8(xT)
        m = dict(shared)
        m["x8h"] = _pair_layout(hi.astype(np.float32)).astype(e4)
        m["x8l"] = _pair_layout(lo.astype(np.float32)).astype(e4)
        m["x16"] = f16c(xT)
        zc = np.stack([colfm(zg_all[b]), colfm(zv_all[b])], axis=1)  # (P,2,ET)
        m["zcols"] = np.ascontiguousarray(zc.reshape(P, 2 * ET), np.float32)
        in_maps.append(m)
    return in_maps, flags


_NC_CACHE = {}


def get_program(reps=1, flags=("skipb",)):
    key = (reps, flags)
    if key not in _NC_CACHE:
        _NC_CACHE[key] = build_program(flags=flags, reps=reps)
    return _NC_CACHE[key]


def kernel(**inputs):
    import os

    from concourse.bass_utils import run_bass_kernel_spmd

    in_maps, flags = prep_inputs(inputs)
    extra = tuple(f for f in os.environ.get("KVAR", "").split(",") if f)
    flags = tuple(flags) + extra
    nc = get_program(flags=flags)
    res = run_bass_kernel_spmd(nc, in_maps, core_ids=list(range(B)))
    return np.stack(
        [np.ascontiguousarray(res.results[b]["out"].T, dtype=np.float32)
         for b in range(B)], axis=1
    )
